# revision 8
# baseline (speedup 1.0000x reference)
"""Channel-attention kernel for Trainium2, data-parallel over batch on 8 NeuronCores.

Reference computation (per batch b):
    xr   = x[b].reshape(HW, C)                  # [4096, 512] fp32
    s    = xr^T @ xr                            # [C, C] gram matrix
    attn = softmax(s, axis=-1)
    v    = xr @ attn                            # [4096, 512]
    out  = beta * v + x[b]

Device strategy (per core: 2 batches, software-pipelined), v5:
  - the host shard step pre-packs three input views, each per-partition
    contiguous in DRAM so every load is a single-segment (cheap-trigger)
    DMA -- multi-segment triggers cost up to 14us of engine time:
      xb  = bf16 natural, half-batch packed   (epilogue, 2 loads/batch)
      xn  = fp8 natural, K-step packed        (GEMM1 operands, 4 loads/batch)
      xt  = fp8 x^T, channel-pair packed      (GEMM2 stationary, 2 loads/batch)
  - DMA-completion semaphore lanes are shared per tile-pool and waits use
    conservative emission-time thresholds, so each batch's input tiles live
    in their own pools (a later batch's in-flight loads must never gate an
    earlier batch's consumers).
  - both GEMMs run fp8 with perf_mode=DoubleRow (virtual 128x256 PE array).
    GEMM1 runs as two half-passes (cb{0,1} then cb{2,3}) so only 2 PSUM
    banks hold gram tiles and the first half's softmax hides under the
    second half's matmuls; each half ends cb-major for the same reason.
  - softmax rows on DVE+ScalarE out of PSUM; beta folded into the
    normalization (attn_scaled = beta * exp(s-max) / sum, written fp8), so
    beta=0 gives v==0 and out = fp32(bf16(x)) exactly.
  - GEMM2 into a 3-deep [128,1024] PSUM ring; the first chunks issue their
    channel-pair-0 matmuls only so the PE restarts before the last softmax
    finishes.  Epilogue (out = v + x): two of three chunks DVE-add straight
    from PSUM, every third is ScalarE-evicted to SBUF and GpSimd-added
    (GpSimd cannot read PSUM); 256-row chunks pair into 512-row stores
    alternating the sync/gpsimd queues.
"""

import ml_dtypes
import numpy as np

import concourse.bass as bass
import concourse.tile as tile
from concourse import bacc, mybir
from concourse.bass_utils import run_bass_kernel_spmd

N_CORES = 8
B_FULL = 16
B_PER_CORE = B_FULL // N_CORES  # 2
H = 64
W = 64
HW = H * W  # 4096
C = 512
NT = HW // 128  # 32 row tiles
CB = C // 128  # 4 channel blocks
NK = NT // 2  # 16 DoubleRow K-steps (256 rows each)
NKC = 4  # xn load chunks per batch (4 K-steps each)
NCH = NT // 2  # 16 GEMM2/epilogue chunks (256 rows each)
NSC = NT // 4  # 8 store super-chunks (512 rows each)
TAILK = 4  # K-steps per half-pass that run cb-major (softmax overlap)
PREFIX = 3  # GEMM2 chunks that issue pair-0 matmuls before pair-1 is ready

F32 = mybir.dt.float32
BF16 = mybir.dt.bfloat16
FP8 = mybir.dt.float8e4
AXL = mybir.AxisListType
ALU = mybir.AluOpType
ACTFN = mybir.ActivationFunctionType
DR = mybir.MatmulPerfMode.DoubleRow


class BatchState:
    def __init__(self):
        self.xbf = []  # 2x [128, 16C] bf16 half-batch tiles (natural layout)
        self.xn8 = []  # 4x [128, 8C] fp8 tiles (4 K-steps each, paired rows)
        self.xt8 = {}  # pair -> [128, 2*HW] fp8 tile (x^T, paired channels)
        self.s_ps = {}  # cb -> [128, C] f32 PSUM
        self.attn = {}  # pair -> [128, 2C] fp8 tile


def emit_input_stage(nc, pools, aps, b, st):
    """Loads for batch b.  Every DMA gets its own single-buffer pool so the
    shared completion-sem thresholds are exact.  fp8 GEMM operands ride the
    scalar queue (earliest deadlines); the bf16 epilogue operands ride the
    slower gpsimd queue (latest deadlines); sync is kept free for stores."""
    xb_ap, xn_ap, xt_ap = aps
    for kc in range(NKC):
        t = pools[f"xn8_{b}_{kc}"].tile(
            [128, 8 * C], FP8, tag="xn8", name=f"xn8_b{b}_k{kc}"
        )
        nc.scalar.dma_start(t[:, :], xn_ap[b, kc, :, :])
        st.xn8.append(t)
    for pair in range(2):
        xt = pools[f"xt8_{b}_{pair}"].tile(
            [128, 2 * HW], FP8, tag="xt8", name=f"xt8_b{b}_p{pair}"
        )
        nc.scalar.dma_start(xt[:, :], xt_ap[b, pair, :, :])
        st.xt8[pair] = xt
    for half in range(2):
        t = pools[f"xbf_{b}_{half}"].tile(
            [128, 16 * C], BF16, tag="xbf", name=f"xbf_b{b}_h{half}"
        )
        nc.gpsimd.dma_start(t[:, :], xb_ap[b, half, :, :])
        st.xbf.append(t)


def emit_g1_step(nc, pools, b, k, cb, st):
    if k == 0:
        st.s_ps[cb] = pools["ps_s"].tile([128, C], F32, tag="s", name=f"s_b{b}_{cb}")
    kc, kl = k // 4, k % 4
    r3 = st.xn8[kc][:, 2 * kl * C : 2 * (kl + 1) * C].rearrange(
        "p (o c) -> p o c", o=2
    )
    nc.tensor.matmul(
        st.s_ps[cb][:, :],
        r3[:, :, cb * 128 : (cb + 1) * 128],
        r3,
        start=(k == 0),
        stop=(k == NK - 1),
        perf_mode=DR,
    )


def emit_softmax(nc, pools, beta_bc, b, cb, st):
    """softmax row block cb out of PSUM -> fp8 half of the attn pair tile."""
    pair, o = cb // 2, cb % 2
    nmax = pools["st"].tile([128, 1], F32, tag="nmax")
    nc.vector.tensor_reduce(
        nmax[:, :], st.s_ps[cb][:, :], axis=AXL.X, op=ALU.max, negate=True
    )
    exps = pools["sm"].tile([128, C], BF16, tag="exps")
    ssum = pools["st"].tile([128, 1], F32, tag="ssum")
    nc.scalar.activation(
        exps[:, :],
        st.s_ps[cb][:, :],
        ACTFN.Exp,
        bias=nmax[:, :],
        scale=1.0,
        accum_out=ssum[:, :],
    )
    rinv = pools["st"].tile([128, 1], F32, tag="rinv")
    nc.vector.reciprocal(rinv[:, :], ssum[:, :])
    rsc = pools["st"].tile([128, 1], F32, tag="rsc")
    nc.vector.tensor_mul(rsc[:, :], rinv[:, :], beta_bc[:, :])
    if o == 0:
        at = pools["attn"].tile(
            [128, 2 * C], FP8, tag="attn", name=f"attn_b{b}_p{pair}"
        )
        st.attn[pair] = at
    nc.scalar.activation(
        st.attn[pair][:, o * C : (o + 1) * C], exps[:, :], ACTFN.Copy, scale=rsc[:, :]
    )


def emit_g1_half(nc, pools, beta_bc, b, half, st):
    """Half-pass over cb pair (2*half, 2*half+1); ends cb-major + softmax."""
    cbs = (2 * half, 2 * half + 1)
    for k in range(NK - TAILK):
        for cb in cbs:
            emit_g1_step(nc, pools, b, k, cb, st)
    for cb in cbs:
        for k in range(NK - TAILK, NK):
            emit_g1_step(nc, pools, b, k, cb, st)
        emit_softmax(nc, pools, beta_bc, b, cb, st)


def emit_g2_mm(nc, b, ch, j, pair, st, vps):
    nt = ch * 2 + j
    xt3 = st.xt8[pair][:, :].rearrange("p (o n) -> p o n", o=2)
    at3 = st.attn[pair][:, :].rearrange("p (o d) -> p o d", o=2)
    nc.tensor.matmul(
        vps[:, j * C : (j + 1) * C],
        xt3[:, :, nt * 128 : (nt + 1) * 128],
        at3,
        start=(pair == 0),
        stop=(pair == 1),
        perf_mode=DR,
    )


def emit_epilogue(nc, pools, out_ap, b, ch, st, vps):
    """out = v + x for chunk ch (row tiles 2ch, 2ch+1), then a 512KB store.
    Every fourth chunk takes the ScalarE-evict + GpSimd-add path to keep
    DVE off the critical path; stores alternate the sync/gpsimd queues."""
    ot = pools["outp"].tile([128, 2 * C], F32, tag="o", name=f"o_b{b}_c{ch}")
    half, off = (2 * ch) // 16, (2 * ch) % 16
    xslice = st.xbf[half][:, off * C : (off + 2) * C]
    if ch % 4 == 1:
        tmp = pools["tmp"].tile([128, 2 * C], F32, tag="tmp")
        nc.scalar.copy(tmp[:, :], vps[:, :])
        nc.gpsimd.tensor_add(ot[:, :], tmp[:, :], xslice)
    else:
        nc.vector.tensor_add(ot[:, :], vps[:, :], xslice)
    eng = nc.sync if ch % 2 == 0 else nc.gpsimd
    eng.dma_start(
        out_ap[b, ch * 256 : (ch + 1) * 256, :].rearrange("(f p) c -> p f c", p=128),
        ot[:, :].rearrange("p (f c) -> p f c", c=C),
    )


def emit_g2(nc, pools, out_ap, b, st):
    vps_ring = {}
    for ch in range(PREFIX):
        vps_ring[ch] = pools["ps_v"].tile(
            [128, 2 * C], F32, tag="v", name=f"v_b{b}_c{ch}"
        )
        for j in range(2):
            emit_g2_mm(nc, b, ch, j, 0, st, vps_ring[ch])
    for ch in range(PREFIX):
        for j in range(2):
            emit_g2_mm(nc, b, ch, j, 1, st, vps_ring[ch])
        emit_epilogue(nc, pools, out_ap, b, ch, st, vps_ring[ch])
    for ch in range(PREFIX, NCH):
        vps = pools["ps_v"].tile([128, 2 * C], F32, tag="v", name=f"v_b{b}_c{ch}")
        for pair in range(2):
            for j in range(2):
                emit_g2_mm(nc, b, ch, j, pair, st, vps)
        emit_epilogue(nc, pools, out_ap, b, ch, st, vps)


def channel_attention_body(tc, out_ap, xb_ap, xn_ap, xt_ap, beta_ap):
    nc = tc.nc
    from contextlib import ExitStack

    with ExitStack() as ctx:
        ep = ctx.enter_context
        pools = {
            "attn": ep(tc.tile_pool(name="attn", bufs=4)),
            "sm": ep(tc.tile_pool(name="sm", bufs=3)),
            "st": ep(tc.tile_pool(name="st", bufs=8)),
            "outp": ep(tc.tile_pool(name="outp", bufs=4)),
            "tmp": ep(tc.tile_pool(name="tmp", bufs=3)),
            "const": ep(tc.tile_pool(name="const", bufs=1)),
            "ps_s": ep(tc.tile_pool(name="ps_s", bufs=2, space="PSUM")),
            "ps_v": ep(tc.tile_pool(name="ps_v", bufs=3, space="PSUM")),
        }
        for b in range(B_PER_CORE):
            for h in range(2):
                pools[f"xbf_{b}_{h}"] = ep(tc.tile_pool(name=f"xbf_{b}_{h}", bufs=1))
            for kc in range(NKC):
                pools[f"xn8_{b}_{kc}"] = ep(tc.tile_pool(name=f"xn8_{b}_{kc}", bufs=1))
            for p in range(2):
                pools[f"xt8_{b}_{p}"] = ep(tc.tile_pool(name=f"xt8_{b}_{p}", bufs=1))

        # beta -> broadcast to [128, 1]
        beta_sb = pools["const"].tile([1, 1], F32, tag="beta")
        nc.sync.dma_start(beta_sb[0:1, 0:1], beta_ap[None, :])
        beta_bc = pools["const"].tile([128, 1], F32, tag="beta_bc")
        nc.gpsimd.partition_broadcast(beta_bc[:, :], beta_sb[0:1, :])

        aps = (xb_ap, xn_ap, xt_ap)
        states = [BatchState() for _ in range(B_PER_CORE)]
        emit_input_stage(nc, pools, aps, 0, states[0])
        for b in range(B_PER_CORE):
            st = states[b]
            emit_g1_half(nc, pools, beta_bc, b, 0, st)
            if b + 1 < B_PER_CORE:
                emit_input_stage(nc, pools, aps, b + 1, states[b + 1])
            emit_g1_half(nc, pools, beta_bc, b, 1, st)
            emit_g2(nc, pools, out_ap, b, st)


_NC_CACHE = None


def _build():
    global _NC_CACHE
    if _NC_CACHE is not None:
        return _NC_CACHE
    nc = bacc.Bacc(
        "TRN2",
        target_bir_lowering=False,
        debug=False,
        num_devices=N_CORES,
    )
    xb_ap = nc.dram_tensor(
        "xb", [B_PER_CORE, 2, 128, 16 * C], BF16, kind="ExternalInput"
    ).ap()
    xn_ap = nc.dram_tensor(
        "xn", [B_PER_CORE, NKC, 128, 8 * C], FP8, kind="ExternalInput"
    ).ap()
    xt_ap = nc.dram_tensor(
        "xt", [B_PER_CORE, 2, 128, 2 * HW], FP8, kind="ExternalInput"
    ).ap()
    beta_ap = nc.dram_tensor("beta", [1], F32, kind="ExternalInput").ap()
    out_ap = nc.dram_tensor(
        "out", [B_PER_CORE, HW, C], F32, kind="ExternalOutput"
    ).ap()
    with tile.TileContext(nc) as tc:
        channel_attention_body(tc, out_ap, xb_ap, xn_ap, xt_ap, beta_ap)
    nc.compile()
    _NC_CACHE = nc
    return nc


def _prep_shard(xr, i):
    """Host-side input prep for core i: every view packed so each DMA is
    per-partition contiguous in DRAM."""
    xs = xr[i * B_PER_CORE : (i + 1) * B_PER_CORE]  # [2, HW, C] fp32
    # xb[b, half, p, f*C+c] = x[b, half*2048 + f*128 + p, c]
    xb = np.ascontiguousarray(
        xs.astype(ml_dtypes.bfloat16)
        .reshape(B_PER_CORE, 2, 16, 128, C)
        .transpose(0, 1, 3, 2, 4)
        .reshape(B_PER_CORE, 2, 128, 16 * C)
    )
    x8 = xs.astype(ml_dtypes.float8_e4m3)
    # xn[b, kc, p, kl, o, c] = x8[b, (kc*4+kl)*256 + o*128 + p, c]
    xn = np.ascontiguousarray(
        x8.reshape(B_PER_CORE, NKC, 4, 2, 128, C)
        .transpose(0, 1, 4, 2, 3, 5)
        .reshape(B_PER_CORE, NKC, 128, 8 * C)
    )
    # xt[b, pair, p, o, n] = x8[b, n, pair*256 + o*128 + p]
    xt = xs.transpose(0, 2, 1).astype(ml_dtypes.float8_e4m3)  # [2, C, HW]
    xt = np.ascontiguousarray(
        xt.reshape(B_PER_CORE, 2, 2, 128, HW)
        .transpose(0, 1, 3, 2, 4)
        .reshape(B_PER_CORE, 2, 128, 2 * HW)
    )
    return xb, xn, xt


def run(x, beta, trace=False, **trace_kwargs):
    """Shard over batch, run on 8 cores, gather. Returns (out, BassKernelResults)."""
    x = np.asarray(x, dtype=np.float32)
    beta = np.asarray(beta, dtype=np.float32)
    assert x.shape == (B_FULL, H, W, C), x.shape
    nc = _build()
    xr = x.reshape(B_FULL, HW, C)
    in_maps = []
    for i in range(N_CORES):
        xb, xn, xt = _prep_shard(xr, i)
        in_maps.append({"xb": xb, "xn": xn, "xt": xt, "beta": beta})
    res = run_bass_kernel_spmd(
        nc, in_maps, core_ids=list(range(N_CORES)), trace=trace, **trace_kwargs
    )
    out = np.concatenate([res.results[i]["out"] for i in range(N_CORES)], axis=0)
    return out.reshape(B_FULL, H, W, C), res


def kernel(x, beta):
    out, _ = run(x, beta, trace=False)
    return out


# revision 9
# speedup vs baseline: 1.0052x; 1.0052x over previous
"""Channel-attention kernel for Trainium2, data-parallel over batch on 8 NeuronCores.

Reference computation (per batch b):
    xr   = x[b].reshape(HW, C)                  # [4096, 512] fp32
    s    = xr^T @ xr                            # [C, C] gram matrix
    attn = softmax(s, axis=-1)
    v    = xr @ attn                            # [4096, 512]
    out  = beta * v + x[b]

Device strategy (per core: 2 batches, software-pipelined), v5:
  - the host shard step pre-packs three input views, each per-partition
    contiguous in DRAM so every load is a single-segment (cheap-trigger)
    DMA -- multi-segment triggers cost up to 14us of engine time:
      xb  = bf16 natural, half-batch packed   (epilogue, 2 loads/batch)\n    and the output is stored bf16 (the host upcasts to fp32): per-core DMA\n    sustains only ~300GB/s aggregate, so bytes are the wall -- bf16 stores\n    halve them at the same ~3e-4-per-element accuracy as the bf16 x load.
      xn  = fp8 natural, K-step packed        (GEMM1 operands, 4 loads/batch)
      xt  = fp8 x^T, channel-pair packed      (GEMM2 stationary, 2 loads/batch)
  - DMA-completion semaphore lanes are shared per tile-pool and waits use
    conservative emission-time thresholds, so each batch's input tiles live
    in their own pools (a later batch's in-flight loads must never gate an
    earlier batch's consumers).
  - both GEMMs run fp8 with perf_mode=DoubleRow (virtual 128x256 PE array).
    GEMM1 runs as two half-passes (cb{0,1} then cb{2,3}) so only 2 PSUM
    banks hold gram tiles and the first half's softmax hides under the
    second half's matmuls; each half ends cb-major for the same reason.
  - softmax rows on DVE+ScalarE out of PSUM; beta folded into the
    normalization (attn_scaled = beta * exp(s-max) / sum, written fp8), so
    beta=0 gives v==0 and out = fp32(bf16(x)) exactly.
  - GEMM2 into a 3-deep [128,1024] PSUM ring; the first chunks issue their
    channel-pair-0 matmuls only so the PE restarts before the last softmax
    finishes.  Epilogue (out = v + x): two of three chunks DVE-add straight
    from PSUM, every third is ScalarE-evicted to SBUF and GpSimd-added
    (GpSimd cannot read PSUM); 256-row chunks pair into 512-row stores
    alternating the sync/gpsimd queues.
"""

import ml_dtypes
import numpy as np

import concourse.bass as bass
import concourse.tile as tile
from concourse import bacc, mybir
from concourse.bass_utils import run_bass_kernel_spmd

N_CORES = 8
B_FULL = 16
B_PER_CORE = B_FULL // N_CORES  # 2
H = 64
W = 64
HW = H * W  # 4096
C = 512
NT = HW // 128  # 32 row tiles
CB = C // 128  # 4 channel blocks
NK = NT // 2  # 16 DoubleRow K-steps (256 rows each)
NKC = 4  # xn load chunks per batch (4 K-steps each)
NCH = NT // 2  # 16 GEMM2/epilogue chunks (256 rows each)
NSC = NT // 4  # 8 store super-chunks (512 rows each)
TAILK = 4  # K-steps per half-pass that run cb-major (softmax overlap)
PREFIX = 3  # GEMM2 chunks that issue pair-0 matmuls before pair-1 is ready

F32 = mybir.dt.float32
BF16 = mybir.dt.bfloat16
FP8 = mybir.dt.float8e4
AXL = mybir.AxisListType
ALU = mybir.AluOpType
ACTFN = mybir.ActivationFunctionType
DR = mybir.MatmulPerfMode.DoubleRow


class BatchState:
    def __init__(self):
        self.xbf = []  # 2x [128, 16C] bf16 half-batch tiles (natural layout)
        self.xn8 = []  # 4x [128, 8C] fp8 tiles (4 K-steps each, paired rows)
        self.xt8 = {}  # pair -> [128, 2*HW] fp8 tile (x^T, paired channels)
        self.s_ps = {}  # cb -> [128, C] f32 PSUM
        self.attn = {}  # pair -> [128, 2C] fp8 tile


def emit_input_stage(nc, pools, aps, b, st):
    """Loads for batch b.  Every DMA gets its own single-buffer pool so the
    shared completion-sem thresholds are exact.  fp8 GEMM operands ride the
    scalar queue (earliest deadlines); the bf16 epilogue operands ride the
    slower gpsimd queue (latest deadlines); sync is kept free for stores."""
    xb_ap, xn_ap, xt_ap = aps
    for kc in range(NKC):
        t = pools[f"xn8_{b}_{kc}"].tile(
            [128, 8 * C], FP8, tag="xn8", name=f"xn8_b{b}_k{kc}"
        )
        nc.scalar.dma_start(t[:, :], xn_ap[b, kc, :, :])
        st.xn8.append(t)
    for pair in range(2):
        xt = pools[f"xt8_{b}_{pair}"].tile(
            [128, 2 * HW], FP8, tag="xt8", name=f"xt8_b{b}_p{pair}"
        )
        nc.sync.dma_start(xt[:, :], xt_ap[b, pair, :, :])
        st.xt8[pair] = xt
    for half in range(2):
        t = pools[f"xbf_{b}_{half}"].tile(
            [128, 16 * C], BF16, tag="xbf", name=f"xbf_b{b}_h{half}"
        )
        nc.gpsimd.dma_start(t[:, :], xb_ap[b, half, :, :])
        st.xbf.append(t)


def emit_g1_step(nc, pools, b, k, cb, st):
    if k == 0:
        st.s_ps[cb] = pools["ps_s"].tile([128, C], F32, tag="s", name=f"s_b{b}_{cb}")
    kc, kl = k // 4, k % 4
    r3 = st.xn8[kc][:, 2 * kl * C : 2 * (kl + 1) * C].rearrange(
        "p (o c) -> p o c", o=2
    )
    nc.tensor.matmul(
        st.s_ps[cb][:, :],
        r3[:, :, cb * 128 : (cb + 1) * 128],
        r3,
        start=(k == 0),
        stop=(k == NK - 1),
        perf_mode=DR,
    )


def emit_softmax(nc, pools, beta_bc, b, cb, st):
    """softmax row block cb out of PSUM -> fp8 half of the attn pair tile."""
    pair, o = cb // 2, cb % 2
    nmax = pools["st"].tile([128, 1], F32, tag="nmax")
    nc.vector.tensor_reduce(
        nmax[:, :], st.s_ps[cb][:, :], axis=AXL.X, op=ALU.max, negate=True
    )
    exps = pools["sm"].tile([128, C], BF16, tag="exps")
    ssum = pools["st"].tile([128, 1], F32, tag="ssum")
    nc.scalar.activation(
        exps[:, :],
        st.s_ps[cb][:, :],
        ACTFN.Exp,
        bias=nmax[:, :],
        scale=1.0,
        accum_out=ssum[:, :],
    )
    rinv = pools["st"].tile([128, 1], F32, tag="rinv")
    nc.vector.reciprocal(rinv[:, :], ssum[:, :])
    rsc = pools["st"].tile([128, 1], F32, tag="rsc")
    nc.vector.tensor_mul(rsc[:, :], rinv[:, :], beta_bc[:, :])
    if o == 0:
        at = pools["attn"].tile(
            [128, 2 * C], FP8, tag="attn", name=f"attn_b{b}_p{pair}"
        )
        st.attn[pair] = at
    nc.scalar.activation(
        st.attn[pair][:, o * C : (o + 1) * C], exps[:, :], ACTFN.Copy, scale=rsc[:, :]
    )


def emit_g1_half(nc, pools, beta_bc, b, half, st):
    """Half-pass over cb pair (2*half, 2*half+1); ends cb-major + softmax."""
    cbs = (2 * half, 2 * half + 1)
    for k in range(NK - TAILK):
        for cb in cbs:
            emit_g1_step(nc, pools, b, k, cb, st)
    for cb in cbs:
        for k in range(NK - TAILK, NK):
            emit_g1_step(nc, pools, b, k, cb, st)
        emit_softmax(nc, pools, beta_bc, b, cb, st)


def emit_g2_mm(nc, b, ch, j, pair, st, vps):
    nt = ch * 2 + j
    xt3 = st.xt8[pair][:, :].rearrange("p (o n) -> p o n", o=2)
    at3 = st.attn[pair][:, :].rearrange("p (o d) -> p o d", o=2)
    nc.tensor.matmul(
        vps[:, j * C : (j + 1) * C],
        xt3[:, :, nt * 128 : (nt + 1) * 128],
        at3,
        start=(pair == 0),
        stop=(pair == 1),
        perf_mode=DR,
    )


def emit_epilogue(nc, pools, out_ap, b, ch, st, vps):
    """out = v + x for chunk ch (row tiles 2ch, 2ch+1), then a 512KB store.
    Every fourth chunk takes the ScalarE-evict + GpSimd-add path to keep
    DVE off the critical path; stores alternate the sync/gpsimd queues."""
    ot = pools["outp"].tile([128, 2 * C], BF16, tag="o", name=f"o_b{b}_c{ch}")
    half, off = (2 * ch) // 16, (2 * ch) % 16
    xslice = st.xbf[half][:, off * C : (off + 2) * C]
    if ch % 4 == 1:
        tmp = pools["tmp"].tile([128, 2 * C], F32, tag="tmp")
        nc.scalar.copy(tmp[:, :], vps[:, :])
        nc.gpsimd.tensor_add(ot[:, :], tmp[:, :], xslice)
    else:
        nc.vector.tensor_add(ot[:, :], vps[:, :], xslice)
    eng = nc.sync if ch % 2 == 0 else nc.gpsimd
    eng.dma_start(
        out_ap[b, ch * 256 : (ch + 1) * 256, :].rearrange("(f p) c -> p f c", p=128),
        ot[:, :].rearrange("p (f c) -> p f c", c=C),
    )


def emit_g2(nc, pools, out_ap, b, st):
    vps_ring = {}
    for ch in range(PREFIX):
        vps_ring[ch] = pools["ps_v"].tile(
            [128, 2 * C], F32, tag="v", name=f"v_b{b}_c{ch}"
        )
        for j in range(2):
            emit_g2_mm(nc, b, ch, j, 0, st, vps_ring[ch])
    for ch in range(PREFIX):
        for j in range(2):
            emit_g2_mm(nc, b, ch, j, 1, st, vps_ring[ch])
        emit_epilogue(nc, pools, out_ap, b, ch, st, vps_ring[ch])
    for ch in range(PREFIX, NCH):
        vps = pools["ps_v"].tile([128, 2 * C], F32, tag="v", name=f"v_b{b}_c{ch}")
        for pair in range(2):
            for j in range(2):
                emit_g2_mm(nc, b, ch, j, pair, st, vps)
        emit_epilogue(nc, pools, out_ap, b, ch, st, vps)


def channel_attention_body(tc, out_ap, xb_ap, xn_ap, xt_ap, beta_ap):
    nc = tc.nc
    from contextlib import ExitStack

    with ExitStack() as ctx:
        ep = ctx.enter_context
        pools = {
            "attn": ep(tc.tile_pool(name="attn", bufs=4)),
            "sm": ep(tc.tile_pool(name="sm", bufs=3)),
            "st": ep(tc.tile_pool(name="st", bufs=8)),
            "outp": ep(tc.tile_pool(name="outp", bufs=6)),
            "tmp": ep(tc.tile_pool(name="tmp", bufs=3)),
            "const": ep(tc.tile_pool(name="const", bufs=1)),
            "ps_s": ep(tc.tile_pool(name="ps_s", bufs=2, space="PSUM")),
            "ps_v": ep(tc.tile_pool(name="ps_v", bufs=3, space="PSUM")),
        }
        for b in range(B_PER_CORE):
            for h in range(2):
                pools[f"xbf_{b}_{h}"] = ep(tc.tile_pool(name=f"xbf_{b}_{h}", bufs=1))
            for kc in range(NKC):
                pools[f"xn8_{b}_{kc}"] = ep(tc.tile_pool(name=f"xn8_{b}_{kc}", bufs=1))
            for p in range(2):
                pools[f"xt8_{b}_{p}"] = ep(tc.tile_pool(name=f"xt8_{b}_{p}", bufs=1))

        # beta -> broadcast to [128, 1]
        beta_sb = pools["const"].tile([1, 1], F32, tag="beta")
        nc.sync.dma_start(beta_sb[0:1, 0:1], beta_ap[None, :])
        beta_bc = pools["const"].tile([128, 1], F32, tag="beta_bc")
        nc.gpsimd.partition_broadcast(beta_bc[:, :], beta_sb[0:1, :])

        aps = (xb_ap, xn_ap, xt_ap)
        states = [BatchState() for _ in range(B_PER_CORE)]
        emit_input_stage(nc, pools, aps, 0, states[0])
        for b in range(B_PER_CORE):
            st = states[b]
            emit_g1_half(nc, pools, beta_bc, b, 0, st)
            if b + 1 < B_PER_CORE:
                emit_input_stage(nc, pools, aps, b + 1, states[b + 1])
            emit_g1_half(nc, pools, beta_bc, b, 1, st)
            emit_g2(nc, pools, out_ap, b, st)


_NC_CACHE = None


def _build():
    global _NC_CACHE
    if _NC_CACHE is not None:
        return _NC_CACHE
    nc = bacc.Bacc(
        "TRN2",
        target_bir_lowering=False,
        debug=False,
        num_devices=N_CORES,
    )
    xb_ap = nc.dram_tensor(
        "xb", [B_PER_CORE, 2, 128, 16 * C], BF16, kind="ExternalInput"
    ).ap()
    xn_ap = nc.dram_tensor(
        "xn", [B_PER_CORE, NKC, 128, 8 * C], FP8, kind="ExternalInput"
    ).ap()
    xt_ap = nc.dram_tensor(
        "xt", [B_PER_CORE, 2, 128, 2 * HW], FP8, kind="ExternalInput"
    ).ap()
    beta_ap = nc.dram_tensor("beta", [1], F32, kind="ExternalInput").ap()
    out_ap = nc.dram_tensor(
        "out", [B_PER_CORE, HW, C], BF16, kind="ExternalOutput"
    ).ap()
    with tile.TileContext(nc) as tc:
        channel_attention_body(tc, out_ap, xb_ap, xn_ap, xt_ap, beta_ap)
    nc.compile()
    _NC_CACHE = nc
    return nc


def _prep_shard(xr, i):
    """Host-side input prep for core i: every view packed so each DMA is
    per-partition contiguous in DRAM."""
    xs = xr[i * B_PER_CORE : (i + 1) * B_PER_CORE]  # [2, HW, C] fp32
    # xb[b, half, p, f*C+c] = x[b, half*2048 + f*128 + p, c]
    xb = np.ascontiguousarray(
        xs.astype(ml_dtypes.bfloat16)
        .reshape(B_PER_CORE, 2, 16, 128, C)
        .transpose(0, 1, 3, 2, 4)
        .reshape(B_PER_CORE, 2, 128, 16 * C)
    )
    x8 = xs.astype(ml_dtypes.float8_e4m3)
    # xn[b, kc, p, kl, o, c] = x8[b, (kc*4+kl)*256 + o*128 + p, c]
    xn = np.ascontiguousarray(
        x8.reshape(B_PER_CORE, NKC, 4, 2, 128, C)
        .transpose(0, 1, 4, 2, 3, 5)
        .reshape(B_PER_CORE, NKC, 128, 8 * C)
    )
    # xt[b, pair, p, o, n] = x8[b, n, pair*256 + o*128 + p]
    xt = xs.transpose(0, 2, 1).astype(ml_dtypes.float8_e4m3)  # [2, C, HW]
    xt = np.ascontiguousarray(
        xt.reshape(B_PER_CORE, 2, 2, 128, HW)
        .transpose(0, 1, 3, 2, 4)
        .reshape(B_PER_CORE, 2, 128, 2 * HW)
    )
    return xb, xn, xt


def run(x, beta, trace=False, **trace_kwargs):
    """Shard over batch, run on 8 cores, gather. Returns (out, BassKernelResults)."""
    x = np.asarray(x, dtype=np.float32)
    beta = np.asarray(beta, dtype=np.float32)
    assert x.shape == (B_FULL, H, W, C), x.shape
    nc = _build()
    xr = x.reshape(B_FULL, HW, C)
    in_maps = []
    for i in range(N_CORES):
        xb, xn, xt = _prep_shard(xr, i)
        in_maps.append({"xb": xb, "xn": xn, "xt": xt, "beta": beta})
    res = run_bass_kernel_spmd(
        nc, in_maps, core_ids=list(range(N_CORES)), trace=trace, **trace_kwargs
    )
    out = np.concatenate(
        [np.asarray(res.results[i]["out"]).astype(np.float32) for i in range(N_CORES)],
        axis=0,
    )
    return out.reshape(B_FULL, H, W, C), res


def kernel(x, beta):
    out, _ = run(x, beta, trace=False)
    return out


# revision 10
# speedup vs baseline: 1.1151x; 1.1093x over previous
"""Channel-attention kernel for Trainium2, data-parallel over batch on 8 NeuronCores.

Reference computation (per batch b):
    xr   = x[b].reshape(HW, C)                  # [4096, 512] fp32
    s    = xr^T @ xr                            # [C, C] gram matrix
    attn = softmax(s, axis=-1)
    v    = xr @ attn                            # [4096, 512]
    out  = beta * v + x[b]

Device strategy (per core: 2 batches, software-pipelined), v5:
  - the host shard step pre-packs three input views, each per-partition
    contiguous in DRAM so every load is a single-segment (cheap-trigger)
    DMA -- multi-segment triggers cost up to 14us of engine time:
      xb  = bf16 natural, half-batch packed   (epilogue, 2 loads/batch)\n    and the output is stored bf16 (the host upcasts to fp32): per-core DMA\n    sustains only ~300GB/s aggregate, so bytes are the wall -- bf16 stores\n    halve them at the same ~3e-4-per-element accuracy as the bf16 x load.
      xn  = fp8 natural, K-step packed        (GEMM1 operands, 4 loads/batch)
      xt  = fp8 x^T, channel-pair packed      (GEMM2 stationary, 2 loads/batch)
  - DMA-completion semaphore lanes are shared per tile-pool and waits use
    conservative emission-time thresholds, so each batch's input tiles live
    in their own pools (a later batch's in-flight loads must never gate an
    earlier batch's consumers).
  - both GEMMs run fp8 with perf_mode=DoubleRow (virtual 128x256 PE array).
    GEMM1 runs as two half-passes (cb{0,1} then cb{2,3}) so only 2 PSUM
    banks hold gram tiles and the first half's softmax hides under the
    second half's matmuls; each half ends cb-major for the same reason.
  - softmax rows on DVE+ScalarE out of PSUM; beta folded into the
    normalization (attn_scaled = beta * exp(s-max) / sum, written fp8), so
    beta=0 gives v==0 and out = fp32(bf16(x)) exactly.
  - GEMM2 into a 3-deep [128,1024] PSUM ring; the first chunks issue their
    channel-pair-0 matmuls only so the PE restarts before the last softmax
    finishes.  Epilogue (out = v + x): two of three chunks DVE-add straight
    from PSUM, every third is ScalarE-evicted to SBUF and GpSimd-added
    (GpSimd cannot read PSUM); 256-row chunks pair into 512-row stores
    alternating the sync/gpsimd queues.
"""

import ml_dtypes
import numpy as np

import concourse.bass as bass
import concourse.tile as tile
from concourse import bacc, mybir
from concourse.bass_utils import run_bass_kernel_spmd

N_CORES = 8
B_FULL = 16
B_PER_CORE = B_FULL // N_CORES  # 2
H = 64
W = 64
HW = H * W  # 4096
C = 512
NT = HW // 128  # 32 row tiles
CB = C // 128  # 4 channel blocks
NK = NT // 2  # 16 DoubleRow K-steps (256 rows each)
NKC = 4  # xn load chunks per batch (4 K-steps each)
NCH = NT // 2  # 16 GEMM2/epilogue chunks (256 rows each)
NSC = NT // 4  # 8 store super-chunks (512 rows each)
TAILK = 4  # K-steps per half-pass that run cb-major (softmax overlap)
PREFIX = 3  # GEMM2 chunks that issue pair-0 matmuls before pair-1 is ready

F32 = mybir.dt.float32
BF16 = mybir.dt.bfloat16
FP8 = mybir.dt.float8e4
AXL = mybir.AxisListType
ALU = mybir.AluOpType
ACTFN = mybir.ActivationFunctionType
DR = mybir.MatmulPerfMode.DoubleRow


class BatchState:
    def __init__(self):
        self.xbf = []  # 2x [128, 16C] bf16 half-batch tiles (natural layout)
        self.xn8 = []  # 4x [128, 8C] fp8 tiles (4 K-steps each, paired rows)
        self.xt8 = {}  # pair -> [128, 2*HW] fp8 tile (x^T, paired channels)
        self.s_ps = {}  # cb -> [128, C] f32 PSUM
        self.attn = {}  # pair -> [128, 2C] fp8 tile


def emit_input_stage(nc, pools, aps, b, st):
    """Loads for batch b.  Every DMA gets its own single-buffer pool so the
    shared completion-sem thresholds are exact.  Engine FIFOs are strict:
    a dependency-free load trigger must never be emitted behind an op that
    waits (it would fire late), so fp8 operands alternate the scalar/sync
    queues, the bf16 epilogue operands ride gpsimd, and stores alternate
    sync/scalar in production order."""
    xb_ap, xn_ap, xt_ap = aps
    for kc in range(NKC):
        t = pools[f"xn8_{b}_{kc}"].tile(
            [128, 8 * C], FP8, tag="xn8", name=f"xn8_b{b}_k{kc}"
        )
        eng = nc.scalar if kc % 2 == 0 else nc.sync
        eng.dma_start(t[:, :], xn_ap[b, kc, :, :])
        st.xn8.append(t)
    for pair in range(2):
        xt = pools[f"xt8_{b}_{pair}"].tile(
            [128, 2 * HW], FP8, tag="xt8", name=f"xt8_b{b}_p{pair}"
        )
        eng = nc.scalar if pair == 0 else nc.sync
        eng.dma_start(xt[:, :], xt_ap[b, pair, :, :])
        st.xt8[pair] = xt
    for half in range(2):
        t = pools[f"xbf_{b}_{half}"].tile(
            [128, 16 * C], BF16, tag="xbf", name=f"xbf_b{b}_h{half}"
        )
        nc.gpsimd.dma_start(t[:, :], xb_ap[b, half, :, :])
        st.xbf.append(t)


def emit_g1_step(nc, pools, b, k, cb, st):
    if k == 0:
        st.s_ps[cb] = pools["ps_s"].tile([128, C], F32, tag="s", name=f"s_b{b}_{cb}")
    kc, kl = k // 4, k % 4
    r3 = st.xn8[kc][:, 2 * kl * C : 2 * (kl + 1) * C].rearrange(
        "p (o c) -> p o c", o=2
    )
    nc.tensor.matmul(
        st.s_ps[cb][:, :],
        r3[:, :, cb * 128 : (cb + 1) * 128],
        r3,
        start=(k == 0),
        stop=(k == NK - 1),
        perf_mode=DR,
    )


def emit_softmax(nc, pools, beta_bc, b, cb, st):
    """softmax row block cb out of PSUM -> fp8 half of the attn pair tile."""
    pair, o = cb // 2, cb % 2
    nmax = pools["st"].tile([128, 1], F32, tag="nmax")
    nc.vector.tensor_reduce(
        nmax[:, :], st.s_ps[cb][:, :], axis=AXL.X, op=ALU.max, negate=True
    )
    exps = pools["sm"].tile([128, C], BF16, tag="exps")
    ssum = pools["st"].tile([128, 1], F32, tag="ssum")
    nc.scalar.activation(
        exps[:, :],
        st.s_ps[cb][:, :],
        ACTFN.Exp,
        bias=nmax[:, :],
        scale=1.0,
        accum_out=ssum[:, :],
    )
    rinv = pools["st"].tile([128, 1], F32, tag="rinv")
    nc.vector.reciprocal(rinv[:, :], ssum[:, :])
    rsc = pools["st"].tile([128, 1], F32, tag="rsc")
    nc.vector.tensor_mul(rsc[:, :], rinv[:, :], beta_bc[:, :])
    if o == 0:
        at = pools["attn"].tile(
            [128, 2 * C], FP8, tag="attn", name=f"attn_b{b}_p{pair}"
        )
        st.attn[pair] = at
    nc.scalar.activation(
        st.attn[pair][:, o * C : (o + 1) * C], exps[:, :], ACTFN.Copy, scale=rsc[:, :]
    )


def emit_g1_half(nc, pools, beta_bc, b, half, st):
    """Half-pass over cb pair (2*half, 2*half+1); ends cb-major + softmax."""
    cbs = (2 * half, 2 * half + 1)
    for k in range(NK - TAILK):
        for cb in cbs:
            emit_g1_step(nc, pools, b, k, cb, st)
    for cb in cbs:
        for k in range(NK - TAILK, NK):
            emit_g1_step(nc, pools, b, k, cb, st)
        emit_softmax(nc, pools, beta_bc, b, cb, st)


def emit_g2_mm(nc, b, ch, j, pair, st, vps):
    nt = ch * 2 + j
    xt3 = st.xt8[pair][:, :].rearrange("p (o n) -> p o n", o=2)
    at3 = st.attn[pair][:, :].rearrange("p (o d) -> p o d", o=2)
    nc.tensor.matmul(
        vps[:, j * C : (j + 1) * C],
        xt3[:, :, nt * 128 : (nt + 1) * 128],
        at3,
        start=(pair == 0),
        stop=(pair == 1),
        perf_mode=DR,
    )


def emit_epilogue(nc, pools, out_ap, b, ch, st, vps):
    """out = v + x for chunk ch (row tiles 2ch, 2ch+1), then a 512KB store.
    Every fourth chunk takes the ScalarE-evict + GpSimd-add path to keep
    DVE off the critical path; stores alternate the sync/gpsimd queues."""
    ot = pools["outp"].tile([128, 2 * C], BF16, tag="o", name=f"o_b{b}_c{ch}")
    half, off = (2 * ch) // 16, (2 * ch) % 16
    xslice = st.xbf[half][:, off * C : (off + 2) * C]
    if ch % 4 == 1:
        tmp = pools["tmp"].tile([128, 2 * C], F32, tag="tmp")
        nc.scalar.copy(tmp[:, :], vps[:, :])
        nc.gpsimd.tensor_add(ot[:, :], tmp[:, :], xslice)
    else:
        nc.vector.tensor_add(ot[:, :], vps[:, :], xslice)
    eng = nc.sync if ch % 2 == 0 else nc.scalar
    eng.dma_start(
        out_ap[b, ch * 256 : (ch + 1) * 256, :].rearrange("(f p) c -> p f c", p=128),
        ot[:, :].rearrange("p (f c) -> p f c", c=C),
    )


def emit_g2(nc, pools, out_ap, b, st):
    vps_ring = {}
    for ch in range(PREFIX):
        vps_ring[ch] = pools["ps_v"].tile(
            [128, 2 * C], F32, tag="v", name=f"v_b{b}_c{ch}"
        )
        for j in range(2):
            emit_g2_mm(nc, b, ch, j, 0, st, vps_ring[ch])
    for ch in range(PREFIX):
        for j in range(2):
            emit_g2_mm(nc, b, ch, j, 1, st, vps_ring[ch])
        emit_epilogue(nc, pools, out_ap, b, ch, st, vps_ring[ch])
    for ch in range(PREFIX, NCH):
        vps = pools["ps_v"].tile([128, 2 * C], F32, tag="v", name=f"v_b{b}_c{ch}")
        for pair in range(2):
            for j in range(2):
                emit_g2_mm(nc, b, ch, j, pair, st, vps)
        emit_epilogue(nc, pools, out_ap, b, ch, st, vps)


def channel_attention_body(tc, out_ap, xb_ap, xn_ap, xt_ap, beta_ap):
    nc = tc.nc
    from contextlib import ExitStack

    with ExitStack() as ctx:
        ep = ctx.enter_context
        pools = {
            "attn": ep(tc.tile_pool(name="attn", bufs=4)),
            "sm": ep(tc.tile_pool(name="sm", bufs=3)),
            "st": ep(tc.tile_pool(name="st", bufs=8)),
            "outp": ep(tc.tile_pool(name="outp", bufs=6)),
            "tmp": ep(tc.tile_pool(name="tmp", bufs=3)),
            "const": ep(tc.tile_pool(name="const", bufs=1)),
            "ps_s": ep(tc.tile_pool(name="ps_s", bufs=2, space="PSUM")),
            "ps_v": ep(tc.tile_pool(name="ps_v", bufs=3, space="PSUM")),
        }
        for b in range(B_PER_CORE):
            for h in range(2):
                pools[f"xbf_{b}_{h}"] = ep(tc.tile_pool(name=f"xbf_{b}_{h}", bufs=1))
            for kc in range(NKC):
                pools[f"xn8_{b}_{kc}"] = ep(tc.tile_pool(name=f"xn8_{b}_{kc}", bufs=1))
            for p in range(2):
                pools[f"xt8_{b}_{p}"] = ep(tc.tile_pool(name=f"xt8_{b}_{p}", bufs=1))

        # beta -> broadcast to [128, 1]
        beta_sb = pools["const"].tile([1, 1], F32, tag="beta")
        nc.sync.dma_start(beta_sb[0:1, 0:1], beta_ap[None, :])
        beta_bc = pools["const"].tile([128, 1], F32, tag="beta_bc")
        nc.gpsimd.partition_broadcast(beta_bc[:, :], beta_sb[0:1, :])

        aps = (xb_ap, xn_ap, xt_ap)
        states = [BatchState() for _ in range(B_PER_CORE)]
        emit_input_stage(nc, pools, aps, 0, states[0])
        for b in range(B_PER_CORE):
            st = states[b]
            emit_g1_half(nc, pools, beta_bc, b, 0, st)
            if b + 1 < B_PER_CORE:
                emit_input_stage(nc, pools, aps, b + 1, states[b + 1])
            emit_g1_half(nc, pools, beta_bc, b, 1, st)
            emit_g2(nc, pools, out_ap, b, st)


_NC_CACHE = None


def _build():
    global _NC_CACHE
    if _NC_CACHE is not None:
        return _NC_CACHE
    nc = bacc.Bacc(
        "TRN2",
        target_bir_lowering=False,
        debug=False,
        num_devices=N_CORES,
    )
    xb_ap = nc.dram_tensor(
        "xb", [B_PER_CORE, 2, 128, 16 * C], BF16, kind="ExternalInput"
    ).ap()
    xn_ap = nc.dram_tensor(
        "xn", [B_PER_CORE, NKC, 128, 8 * C], FP8, kind="ExternalInput"
    ).ap()
    xt_ap = nc.dram_tensor(
        "xt", [B_PER_CORE, 2, 128, 2 * HW], FP8, kind="ExternalInput"
    ).ap()
    beta_ap = nc.dram_tensor("beta", [1], F32, kind="ExternalInput").ap()
    out_ap = nc.dram_tensor(
        "out", [B_PER_CORE, HW, C], BF16, kind="ExternalOutput"
    ).ap()
    with tile.TileContext(nc) as tc:
        channel_attention_body(tc, out_ap, xb_ap, xn_ap, xt_ap, beta_ap)
    nc.compile()
    _NC_CACHE = nc
    return nc


def _prep_shard(xr, i):
    """Host-side input prep for core i: every view packed so each DMA is
    per-partition contiguous in DRAM."""
    xs = xr[i * B_PER_CORE : (i + 1) * B_PER_CORE]  # [2, HW, C] fp32
    # xb[b, half, p, f*C+c] = x[b, half*2048 + f*128 + p, c]
    xb = np.ascontiguousarray(
        xs.astype(ml_dtypes.bfloat16)
        .reshape(B_PER_CORE, 2, 16, 128, C)
        .transpose(0, 1, 3, 2, 4)
        .reshape(B_PER_CORE, 2, 128, 16 * C)
    )
    x8 = xs.astype(ml_dtypes.float8_e4m3)
    # xn[b, kc, p, kl, o, c] = x8[b, (kc*4+kl)*256 + o*128 + p, c]
    xn = np.ascontiguousarray(
        x8.reshape(B_PER_CORE, NKC, 4, 2, 128, C)
        .transpose(0, 1, 4, 2, 3, 5)
        .reshape(B_PER_CORE, NKC, 128, 8 * C)
    )
    # xt[b, pair, p, o, n] = x8[b, n, pair*256 + o*128 + p]
    xt = xs.transpose(0, 2, 1).astype(ml_dtypes.float8_e4m3)  # [2, C, HW]
    xt = np.ascontiguousarray(
        xt.reshape(B_PER_CORE, 2, 2, 128, HW)
        .transpose(0, 1, 3, 2, 4)
        .reshape(B_PER_CORE, 2, 128, 2 * HW)
    )
    return xb, xn, xt


def run(x, beta, trace=False, **trace_kwargs):
    """Shard over batch, run on 8 cores, gather. Returns (out, BassKernelResults)."""
    x = np.asarray(x, dtype=np.float32)
    beta = np.asarray(beta, dtype=np.float32)
    assert x.shape == (B_FULL, H, W, C), x.shape
    nc = _build()
    xr = x.reshape(B_FULL, HW, C)
    in_maps = []
    for i in range(N_CORES):
        xb, xn, xt = _prep_shard(xr, i)
        in_maps.append({"xb": xb, "xn": xn, "xt": xt, "beta": beta})
    res = run_bass_kernel_spmd(
        nc, in_maps, core_ids=list(range(N_CORES)), trace=trace, **trace_kwargs
    )
    out = np.concatenate(
        [np.asarray(res.results[i]["out"]).astype(np.float32) for i in range(N_CORES)],
        axis=0,
    )
    return out.reshape(B_FULL, H, W, C), res


def kernel(x, beta):
    out, _ = run(x, beta, trace=False)
    return out
